# revision 10
# baseline (speedup 1.0000x reference)
"""Trainium2 Bass kernel: sigmoid(rowdot(tanh(x1@W.T+b), tanh(x2@W.T+b))).

Sharding: pure data-parallel over batch across 8 NeuronCores (B=65536
total -> 8192 rows/core, D_IN=1024, D_PROJ=128).

The kernel is DMA-bound on the activation loads, so the host pre-packs
x1/x2 into fp16 (end-to-end max rel err ~5e-3 vs the 2e-2 gate, measured
on the reference distribution) and into the exact PE-ready transposed
tile layout, halving HBM traffic to 32 MiB/core (~86 us at the measured
~394 GB/s per-NC DMA rate, which is the 16-SDMA-engine limit) and
eliminating every on-device PE transpose. Per-core flat layout (one
dram tensor, contiguous per work item):

  big item t (1024 rows, t=0..6), segment at t*16384:
      [p, s*8192 + k*1024 + b] = xs[t*1024 + b, k*128 + p]
  small items (512 rows, h=0,1), segments at 114688 + h*8192:
      [p, s*4096 + k*512 + b]  = xs[7168 + h*512 + b, k*128 + p]

Each big item is ONE contiguous 4 MiB DMA; chunks land
contraction-on-partitions, ready to be the matmul moving operand
(N=1024 fp16). The final small item loads x1 whole then x2 split
k0-5/k6/k7, so the post-last-byte drain is a single N=512 matmul +
tanh/mul/reduce/sigmoid + one 2 KiB store (~5 us).

Per item: fp16 matmuls accumulate oT=W.T@xT chunkwise into PSUM; ACT
fuses tanh(po+bias) PSUM->SBUF; DVE multiplies; PE reduces partitions
via ones[128,128] matmul (f32r, N<=512 per op); ACT sigmoid; small
store from a rotating partition. The partition reduce is deferred into
the next item's first matmul group so PE never waits on the tanh->mul
chain. Work items are as large as PSUM allows (po1/po2/psim of 2 banks
x 4 pool buffers = all 8 banks) because every item/DMA/cross-engine
edge burns a semaphore and the framework teardown resets each allocated
semaphore individually (~115 ns/5-engine round on the critical path).
"""

import numpy as np

import concourse.bacc as bacc
import concourse.mybir as mybir
import concourse.tile as tile
from concourse.bass_utils import run_bass_kernel_spmd

N_CORES = 8
B_TOTAL = 65536
BSH = B_TOTAL // N_CORES  # 8192 rows per core
D_IN = 1024
D_PROJ = 128
P = 128
KC = D_IN // P            # 8 contraction chunks
BT1 = 1024                # big-item batch rows (fp16 moving-operand max)
NB1 = 7                   # big items
BT2 = 512                 # small-item batch rows
NB2 = 2                   # small items
SEG1 = 2 * KC * BT1       # 16384 free elems per big item (x1|x2)
SEG2 = 2 * KC * BT2       # 8192 per small item
TOT = NB1 * SEG1 + NB2 * SEG2  # 131072 free elems per core

F32 = mybir.dt.float32
F32R = mybir.dt.float32r
F16 = mybir.dt.float16

# (offset, row0, nrows) per work item
ITEMS = [(t * SEG1, t * BT1, BT1) for t in range(NB1)]
ITEMS += [(NB1 * SEG1 + h * SEG2, NB1 * BT1 + h * BT2, BT2) for h in range(NB2)]


def _build_module():
    nc = bacc.Bacc("TRN2", target_bir_lowering=False, debug=False)

    xc = nc.dram_tensor("xc", [P, TOT], F16, kind="ExternalInput").ap()
    wt = nc.dram_tensor("wt", [P, KC, D_PROJ], F16, kind="ExternalInput").ap()
    bias = nc.dram_tensor("bias", [P, 1], F32, kind="ExternalInput").ap()
    ones = nc.dram_tensor("ones", [P, P], F32R, kind="ExternalInput").ap()
    out = nc.dram_tensor("out", [BSH], F32, kind="ExternalOutput").ap()

    with tile.TileContext(nc) as tc:
        with (
            tc.tile_pool(name="consts", bufs=1) as cpool,
            tc.tile_pool(name="x", bufs=3) as xpool,
            tc.tile_pool(name="acts", bufs=2) as apool,
            tc.tile_pool(name="po", bufs=4, space="PSUM") as opool,
        ):
            wt_sb = cpool.tile([P, KC, D_PROJ], F16, tag="wt")
            bias_sb = cpool.tile([P, 1], F32, tag="bias")
            ones_sb = cpool.tile([P, P], F32R, tag="ones")

            pending = []

            def flush_pending():
                while pending:
                    prod_p, row0_p, nr_p, idx_p = pending.pop(0)
                    psim = opool.tile([P, nr_p], F32, name="psim", tag="po")
                    for c0 in range(0, nr_p, BT2):  # f32r moving max 512
                        nc.tensor.matmul(
                            psim[:, c0:c0 + BT2],
                            ones_sb,
                            prod_p[:, c0:c0 + BT2],
                            start=True,
                            stop=True,
                            skip_group_check=True,
                        )
                    sig = apool.tile([P, nr_p], F32, tag="sig")
                    nc.scalar.activation(
                        sig, psim, mybir.ActivationFunctionType.Sigmoid
                    )
                    row = (idx_p * 8) % P  # rotate partition -> spread DMA engines
                    nc.scalar.dma_start(
                        out=out[row0_p:row0_p + nr_p].rearrange(
                            "(a n) -> a n", a=1
                        ),
                        in_=sig[row:row + 1, :],
                    )

            def mm_group(sb, nrows, tens, mid=None):
                base = tens * KC * nrows
                po = opool.tile([P, nrows], F32, name=f"po{tens}", tag="po")
                for k in range(KC):
                    # fp32 matmul output is capped at 512 (one PSUM bank);
                    # wide items emit per-bank sub-matmuls.
                    for c0 in range(0, nrows, BT2):
                        nc.tensor.matmul(
                            po[:, c0:c0 + min(BT2, nrows)],
                            wt_sb[:, k, :],
                            sb[:, base + k * nrows + c0:
                               base + k * nrows + c0 + min(BT2, nrows)],
                            start=(k == 0),
                            stop=(k == KC - 1),
                            skip_group_check=True,
                        )
                    if k == 2 and mid is not None:
                        mid()
                t_sb = apool.tile([P, nrows], F32, tag=f"t{tens}")
                nc.scalar.activation(
                    t_sb, po, mybir.ActivationFunctionType.Tanh, bias=bias_sb
                )
                return t_sb

            loaded = {}

            def load(j):
                off, _, nrows = ITEMS[j]
                seg = 2 * KC * nrows
                sb = xpool.tile([P, seg], F16, tag="sb")
                if j < len(ITEMS) - 1:
                    nc.sync.dma_start(out=sb, in_=xc[:, off:off + seg])
                else:
                    # Final item: x1 whole, then x2 split k0-5 / k6 / k7 so
                    # the post-last-byte chain is one N=512 matmul + tail.
                    hw = seg // 2
                    nc.sync.dma_start(out=sb[:, :hw], in_=xc[:, off:off + hw])
                    cuts = [0, 6 * BT2, 7 * BT2, 8 * BT2]
                    for a, b in zip(cuts[:-1], cuts[1:]):
                        nc.sync.dma_start(
                            out=sb[:, hw + a:hw + b],
                            in_=xc[:, off + hw + a:off + hw + b],
                        )
                loaded[j] = sb

            def compute(j):
                _, row0, nrows = ITEMS[j]
                sb = loaded.pop(j)
                # pending reduce of the previous item rides between the
                # two matmul groups so PE never waits on tanh->mul.
                t1 = mm_group(sb, nrows, 0, mid=flush_pending)
                t2 = mm_group(sb, nrows, 1)
                prod = apool.tile([P, nrows], F32R, tag="prod")
                nc.vector.tensor_mul(prod, t1, t2)
                pending.append((prod, row0, nrows, j))

            # Issue order: x item 0 first on the sync ring (it IS the
            # stream bottleneck); wt/bias/ones ride the otherwise-idle
            # scalar ring (wt gates the first matmul, bias the first
            # tanh, ones the first reduce).
            load(0)
            nc.scalar.dma_start(out=wt_sb, in_=wt)
            nc.scalar.dma_start(out=bias_sb, in_=bias)
            nc.scalar.dma_start(out=ones_sb, in_=ones)
            for j in range(1, len(ITEMS)):
                load(j)
                compute(j - 1)
            compute(len(ITEMS) - 1)
            flush_pending()

    nc.compile()
    return nc


_NC_CACHE = None


def _get_module():
    global _NC_CACHE
    if _NC_CACHE is None:
        _NC_CACHE = _build_module()
    return _NC_CACHE


def _pack_x(x, flat, s):
    """Pack one input tensor into its half of the per-item segments.

    x: [B, D_IN] fp32; flat: [N_CORES, P, TOT] fp16 (out); s: 0 for x1,
    1 for x2.
    """
    xh = np.asarray(x, dtype=np.float32).astype(np.float16)
    xh = xh.reshape(N_CORES, BSH, D_IN)
    # big items: [c, t, b, k, p] -> [c, t, p, k, b]
    big = xh[:, :NB1 * BT1].reshape(N_CORES, NB1, BT1, KC, P)
    big = big.transpose(0, 1, 4, 3, 2).reshape(N_CORES, NB1, P, KC * BT1)
    for t in range(NB1):
        o = t * SEG1 + s * KC * BT1
        flat[:, :, o:o + KC * BT1] = big[:, t]
    # small items
    sm = xh[:, NB1 * BT1:].reshape(N_CORES, NB2, BT2, KC, P)
    sm = sm.transpose(0, 1, 4, 3, 2).reshape(N_CORES, NB2, P, KC * BT2)
    for h in range(NB2):
        o = NB1 * SEG1 + h * SEG2 + s * KC * BT2
        flat[:, :, o:o + KC * BT2] = sm[:, h]


def _pack_inputs(x1, x2, W, b):
    flat = np.empty((N_CORES, P, TOT), dtype=np.float16)
    _pack_x(x1, flat, 0)
    _pack_x(x2, flat, 1)
    wt = np.ascontiguousarray(
        np.asarray(W, dtype=np.float32).T.reshape(KC, P, D_PROJ)
        .transpose(1, 0, 2)
    ).astype(np.float16)
    bias = np.ascontiguousarray(np.asarray(b, dtype=np.float32).reshape(P, 1))
    ones = np.ones((P, P), dtype=np.float32)
    return [
        {
            "xc": np.ascontiguousarray(flat[i]),
            "wt": wt,
            "bias": bias,
            "ones": ones,
        }
        for i in range(N_CORES)
    ]


def kernel(x1, x2, W, b):
    nc = _get_module()
    in_maps = _pack_inputs(x1, x2, W, b)
    res = run_bass_kernel_spmd(nc, in_maps, core_ids=list(range(N_CORES)))
    return np.concatenate([res.results[i]["out"] for i in range(N_CORES)])


# revision 11
# speedup vs baseline: 1.0753x; 1.0753x over previous
"""Trainium2 Bass kernel: sigmoid(rowdot(tanh(x1@W.T+b), tanh(x2@W.T+b))).

Sharding: pure data-parallel over batch across 8 NeuronCores (B=65536
total -> 8192 rows/core, D_IN=1024, D_PROJ=128).

The kernel is DMA-bound on the activation loads, so the host pre-packs
x1/x2 into fp16 (end-to-end max rel err ~5e-3 vs the 2e-2 gate, measured
on the reference distribution) and into the exact PE-ready transposed
tile layout, halving HBM traffic to 32 MiB/core (~86 us at the measured
~394 GB/s per-NC DMA rate, which is the 16-SDMA-engine limit) and
eliminating every on-device PE transpose:

  xc[t][p, s*4096 + k*BT + b] = xs[t*BT + b, k*128 + p]   (s=0: x1, 1: x2)

Each 512-row tile is loaded by TWO 1 MiB contiguous DMAs (x1 half, x2
half) so the matmuls on x1 start as soon as that half's completion
semaphore fires — one fused DMA per tile would leave PE a full tile
(~5 us) behind the stream, which materializes as drain at the end. The
last tile's x2 half is further split k0-5/k6/k7 so the post-last-byte
critical path is one N=512 matmul + tanh/mul/reduce/sigmoid + one 2 KiB
store (~5 us total drain).

Per tile: 8 fp16 matmuls (N=512, 1 cyc/row warm at 2.4 GHz) accumulate
oT=W.T@xT chunkwise into one PSUM bank; ACT fuses tanh(po+bias)
PSUM->SBUF; same for x2; DVE multiplies; PE reduces partitions via
ones[128,128] matmul (f32r); ACT sigmoid; 2 KiB store from a rotating
partition. PE load is ~4.2 us/tile vs ~5.2 us/tile of DMA, so only the
partition reduce needs manual deferral (emitted between the next tile's
two matmul groups) to avoid an in-order PE stall behind the tanh->mul
chain. wt/bias/ones ride the scalar-engine DMA ring so the sync ring is
purely the x stream; output stores also use the scalar ring.
"""

import numpy as np

import concourse.bacc as bacc
import concourse.mybir as mybir
import concourse.tile as tile
from concourse.bass_utils import run_bass_kernel_spmd

N_CORES = 8
B_TOTAL = 65536
BSH = B_TOTAL // N_CORES  # 8192 rows per core
D_IN = 1024
D_PROJ = 128
P = 128
BT = 512                 # batch tile (matmul moving dim)
NBT = BSH // BT          # 16 batch tiles per core
KC = D_IN // P           # 8 contraction chunks
FW = KC * BT             # 4096 free-dim elements per packed half-tile

F32 = mybir.dt.float32
F32R = mybir.dt.float32r
F16 = mybir.dt.float16


def _build_module():
    nc = bacc.Bacc("TRN2", target_bir_lowering=False, debug=False)

    xc = nc.dram_tensor("xc", [NBT, P, 2 * FW], F16, kind="ExternalInput").ap()
    wt = nc.dram_tensor("wt", [P, KC, D_PROJ], F16, kind="ExternalInput").ap()
    bias = nc.dram_tensor("bias", [P, 1], F32, kind="ExternalInput").ap()
    ones = nc.dram_tensor("ones", [P, P], F32R, kind="ExternalInput").ap()
    out = nc.dram_tensor("out", [BSH], F32, kind="ExternalOutput").ap()

    with tile.TileContext(nc) as tc:
        with (
            tc.tile_pool(name="consts", bufs=1) as cpool,
            tc.tile_pool(name="x", bufs=3) as xpool,
            tc.tile_pool(name="acts", bufs=2) as apool,
            tc.tile_pool(name="po", bufs=6, space="PSUM") as opool,
        ):
            wt_sb = cpool.tile([P, KC, D_PROJ], F16, tag="wt")
            bias_sb = cpool.tile([P, 1], F32, tag="bias")
            ones_sb = cpool.tile([P, P], F32R, tag="ones")

            pending = []

            def flush_pending():
                while pending:
                    prod_p, row0_p, idx_p = pending.pop(0)
                    psim = opool.tile([P, BT], F32, name="psim", tag="po")
                    nc.tensor.matmul(
                        psim,
                        ones_sb,
                        prod_p,
                        start=True,
                        stop=True,
                        skip_group_check=True,
                    )
                    sig = apool.tile([P, BT], F32, tag="sig")
                    nc.scalar.activation(
                        sig, psim, mybir.ActivationFunctionType.Sigmoid
                    )
                    row = (idx_p * 4) % P  # rotate partition -> spread DMA engines
                    nc.scalar.dma_start(
                        out=out[row0_p:row0_p + BT].rearrange(
                            "(a n) -> a n", a=1
                        ),
                        in_=sig[row:row + 1, :],
                    )

            def mm_group(sb, tens, mid=None):
                base = tens * FW
                po = opool.tile([P, BT], F32, name=f"po{tens}", tag="po")
                for k in range(KC):
                    nc.tensor.matmul(
                        po,
                        wt_sb[:, k, :],
                        sb[:, base + k * BT:base + (k + 1) * BT],
                        start=(k == 0),
                        stop=(k == KC - 1),
                        skip_group_check=True,
                    )
                    if k == 2 and mid is not None:
                        mid()
                t_sb = apool.tile([P, BT], F32, tag=f"t{tens}")
                nc.scalar.activation(
                    t_sb, po, mybir.ActivationFunctionType.Tanh, bias=bias_sb
                )
                return t_sb

            loaded = {}

            def load(t):
                sb = xpool.tile([P, 2 * FW], F16, tag="sb")
                nc.sync.dma_start(out=sb[:, :FW], in_=xc[t][:, :FW])
                if t < NBT - 1:
                    nc.sync.dma_start(out=sb[:, FW:], in_=xc[t][:, FW:])
                else:
                    # Final tile: x2 split k0-5 / k6 / k7 so the
                    # post-last-byte chain is one N=512 matmul + tail.
                    cuts = [0, 6 * BT, 7 * BT, 8 * BT]
                    for a, b in zip(cuts[:-1], cuts[1:]):
                        nc.sync.dma_start(
                            out=sb[:, FW + a:FW + b],
                            in_=xc[t][:, FW + a:FW + b],
                        )
                loaded[t] = sb

            def compute(t):
                sb = loaded.pop(t)
                # pending reduce of the previous tile rides between the
                # two matmul groups so PE never waits on tanh->mul.
                t1 = mm_group(sb, 0, mid=flush_pending)
                t2 = mm_group(sb, 1)
                prod = apool.tile([P, BT], F32R, tag="prod")
                nc.vector.tensor_mul(prod, t1, t2)
                pending.append((prod, t * BT, t))

            # Issue order: x tile 0 first on the sync ring (it IS the
            # stream bottleneck); wt/bias/ones ride the otherwise-idle
            # scalar ring (wt gates the first matmul, bias the first
            # tanh, ones the first reduce).
            load(0)
            nc.scalar.dma_start(out=wt_sb, in_=wt)
            nc.scalar.dma_start(out=bias_sb, in_=bias)
            nc.scalar.dma_start(out=ones_sb, in_=ones)
            for t in range(1, NBT):
                load(t)
                compute(t - 1)
            compute(NBT - 1)
            flush_pending()

    nc.compile()
    return nc


_NC_CACHE = None


def _get_module():
    global _NC_CACHE
    if _NC_CACHE is None:
        _NC_CACHE = _build_module()
    return _NC_CACHE


def _pack_x(x):
    """[B, D_IN] fp32 -> [N_CORES, NBT, P, FW] fp16 PE-ready tiles.

    Slot t holds tile t's transposed layout [p, k*BT + b].
    """
    xh = np.asarray(x, dtype=np.float32).astype(np.float16)
    a = xh.reshape(N_CORES, NBT, BT, KC, P).transpose(0, 1, 4, 3, 2)
    return np.ascontiguousarray(a).reshape(N_CORES, NBT, P, FW)


def _pack_inputs(x1, x2, W, b):
    f1 = _pack_x(x1)
    f2 = _pack_x(x2)
    xc_all = np.concatenate([f1, f2], axis=3)  # [c, t, p, 2*FW]
    wt = np.ascontiguousarray(
        np.asarray(W, dtype=np.float32).T.reshape(KC, P, D_PROJ)
        .transpose(1, 0, 2)
    ).astype(np.float16)
    bias = np.ascontiguousarray(np.asarray(b, dtype=np.float32).reshape(P, 1))
    ones = np.ones((P, P), dtype=np.float32)
    return [
        {
            "xc": np.ascontiguousarray(xc_all[i]),
            "wt": wt,
            "bias": bias,
            "ones": ones,
        }
        for i in range(N_CORES)
    ]


def kernel(x1, x2, W, b):
    nc = _get_module()
    in_maps = _pack_inputs(x1, x2, W, b)
    res = run_bass_kernel_spmd(nc, in_maps, core_ids=list(range(N_CORES)))
    return np.concatenate([res.results[i]["out"] for i in range(N_CORES)])
